# revision 34
# baseline (speedup 1.0000x reference)
"""CanineEmbeddings (multi-hash bucket embedding lookup + LayerNorm) on 8 TRN2 cores.

Key observation: every bucket hash ((id+1)*prime_h) % 16384 depends only on
m = (id+1) mod 16384, so there are exactly 16384 distinct embedding vectors.
The host fuses the 8 bucket tables into one table F[m] = concat_h T_h[(m*p_h)%16384]
(pure weight preprocessing), and the device does ONE 3072-byte dma_gather per
token instead of eight 384-byte ones.

Per-core structure (data-parallel over batch; one 8192-token row per core):
  - ids arrive wrapped-16 ([p, g, s] = id[g*1024 + s*16 + p%16], replicated
    across the 8 gpsimd core groups) so the SWDGE gather can read them.
  - idx = (id & 16383) + 1 on DVE (2 ops); F has 16385 rows with row 16384
    aliasing row 0 so the +1 never needs a second mod.
  - per 1024-token group: one dma_gather (SWDGE 'mlp' Q7 library, 4 queues
    round-robin) -> gt[p, chunk, 768] is already the packed layout.
  - LayerNorm per 128-token chunk: 2x bn_stats (384 elems each) + bn_aggr +
    sqrt + reciprocal; normalize in-place, alternating ACT and DVE.
  - store per 2 chunks (HWDGE, 3072B descriptors).
"""

import contextlib
import ctypes
import os
import sys
import types

import numpy as np

import concourse.bacc as bacc
import concourse.bass as bass
import concourse.mybir as mybir
import concourse.tile as tile
from concourse.bass_utils import run_bass_kernel_spmd
from concourse.library_config import mlp as _mlp_lib
from concourse.tile import add_dep_helper


def _ensure_axon_ntff_hook():
    """The agent image's ``antenv`` lacks ``axon_hooks``; provide it (and the
    ctypes NTFF profile hook) so run_bass_kernel_spmd(trace=True) works.
    Degrades to a None hook (no trace, run still works) on any failure."""
    if "antenv.axon_hooks" in sys.modules:
        return
    hook = None
    try:
        so_path = "/opt/axon/libaxon_pjrt.so"
        lib = ctypes.CDLL(so_path)
        if hasattr(lib, "axon_start_nrt_profile"):
            lib.axon_start_nrt_profile.argtypes = [
                ctypes.POINTER(ctypes.c_int64),
                ctypes.c_size_t,
            ]
            lib.axon_start_nrt_profile.restype = ctypes.c_int64
            lib.axon_stop_nrt_profile.argtypes = [ctypes.c_char_p]
            lib.axon_stop_nrt_profile.restype = ctypes.c_int64

            @contextlib.contextmanager
            def _hook(output_dir, device_ids):
                import jax

                jax.devices()
                if device_ids:
                    ids = (ctypes.c_int64 * len(device_ids))(*device_ids)
                    rc = lib.axon_start_nrt_profile(ids, len(device_ids))
                else:
                    rc = lib.axon_start_nrt_profile(None, 0)
                if rc != 0:
                    raise RuntimeError(f"axon_start_nrt_profile rc={rc}")
                try:
                    yield
                finally:
                    n = lib.axon_stop_nrt_profile(str(output_dir).encode())
                    print(f"ntff profile: {n} file(s) -> {output_dir}", file=sys.stderr)

            hook = _hook
    except Exception as e:  # pragma: no cover
        print(f"ntff hook unavailable: {e}", file=sys.stderr)
    mod = types.ModuleType("antenv.axon_hooks")
    mod.get_axon_ntff_profile_hook = lambda: hook
    mod.set_axon_ntff_profile_hook = lambda h: None
    sys.modules["antenv.axon_hooks"] = mod


_ensure_axon_ntff_hook()

PRIMES = [31, 43, 59, 61, 73, 97, 103, 113]
NUM_HASHES = 8
NUM_BUCKETS = 16384
HIDDEN = 768
SHARD = 96
LN_EPS = 1e-6
N_CORES = 8
GROUP = 1024  # tokens per gather (SWDGE ring caps one gather at 1024 descriptors)
CHUNK = 128  # tokens per LayerNorm chunk (one partition sweep)

AluOp = mybir.AluOpType
Act = mybir.ActivationFunctionType


def _build(tok_per_core: int, affine: bool, enable_asserts: bool = False):
    n_groups = tok_per_core // GROUP
    n_chunks = GROUP // CHUNK  # 8
    wrap_s = GROUP // 16  # 64
    f32, i32, i16 = mybir.dt.float32, mybir.dt.int32, mybir.dt.int16

    nc = bacc.Bacc(
        "TRN2",
        target_bir_lowering=False,
        debug=False,
        enable_asserts=enable_asserts,
        # dma_gather desc-gen runs on the Q7 cpu pair selected by queue_num;
        # 4 queues let up to 4 gathers generate descriptors concurrently.
        num_swdge_queues=4,
    )

    ids_d = nc.dram_tensor("ids", [128, n_groups * wrap_s], i32, kind="ExternalInput")
    ftab_d = nc.dram_tensor(
        "ftab", [NUM_BUCKETS + 1, HIDDEN], f32, kind="ExternalInput"
    )
    out_d = nc.dram_tensor("out", [tok_per_core, HIDDEN], f32, kind="ExternalOutput")
    if affine:
        sc_d = nc.dram_tensor("ln_scale", [128, HIDDEN], f32, kind="ExternalInput")
        bi_d = nc.dram_tensor("ln_bias", [128, HIDDEN], f32, kind="ExternalInput")

    from contextlib import ExitStack

    with tile.TileContext(nc) as tc, ExitStack() as ctx:
        # dma_gather is a Q7 extended instruction living in the 'mlp' ucode
        # library; it must be loaded on the Pool engine before any gather.
        lib_inst = nc.gpsimd.load_library(_mlp_lib).ins

        const = ctx.enter_context(tc.tile_pool(name="const", bufs=1))
        gpool = ctx.enter_context(tc.tile_pool(name="gather", bufs=4))
        spool = ctx.enter_context(tc.tile_pool(name="stats", bufs=8))

        eps_sb = const.tile([128, 1], f32)
        nc.vector.memset(eps_sb[:], LN_EPS)

        ids_sb = const.tile([128, n_groups, wrap_s], i32)
        nc.sync.dma_start(
            out=ids_sb[:],
            in_=ids_d[:].rearrange("p (g s) -> p g s", g=n_groups),
        )
        if affine:
            sc_sb = const.tile([128, HIDDEN], f32)
            nc.sync.dma_start(out=sc_sb[:], in_=sc_d[:])
            bi_sb = const.tile([128, HIDDEN], f32)
            nc.sync.dma_start(out=bi_sb[:], in_=bi_d[:])

        # idx = (id & 16383) + 1 in [1, 16384]; F row 16384 aliases row 0.
        # DVE arithmetic runs in fp32, but all values stay < 2^24 so exact.
        m_sb = const.tile([128, n_groups, wrap_s], i32)
        nc.vector.tensor_scalar(
            out=m_sb[:],
            in0=ids_sb[:],
            scalar1=NUM_BUCKETS - 1,
            scalar2=None,
            op0=AluOp.bitwise_and,
        )
        idx_all = const.tile([128, n_groups, wrap_s], i16)
        nc.vector.tensor_scalar(
            out=idx_all[:],
            in0=m_sb[:],
            scalar1=1,
            scalar2=None,
            op0=AluOp.add,
        )

        for g in range(n_groups):
            # gt[p, chunk, 0:768]: token (g*1024 + chunk*128 + p)'s full
            # (pre-LayerNorm) embedding, gathered in packed layout.
            gt = gpool.tile([128, n_chunks, HIDDEN], f32)
            gi = nc.gpsimd.dma_gather(
                out_ap=gt[:],
                in_ap=ftab_d[:],
                idxs_ap=idx_all[:, g, :],
                num_idxs=GROUP,
                num_idxs_reg=GROUP,
                elem_size=HIDDEN,
                queue_num=g % 4,
            )
            add_dep_helper(gi.ins, lib_inst, sync=False, reason="needs mlp lib")

            for c in range(n_chunks):
                stats = spool.tile([128, 2, 6], f32)
                nc.vector.bn_stats(out=stats[:, 0, :], in_=gt[:, c, 0 : HIDDEN // 2])
                nc.vector.bn_stats(out=stats[:, 1, :], in_=gt[:, c, HIDDEN // 2 :])
                mv = spool.tile([128, 2], f32)
                nc.vector.bn_aggr(out=mv[:], in_=stats[:])
                sd = spool.tile([128, 1], f32)
                nc.scalar.activation(
                    out=sd[:], in_=mv[:, 1:2], func=Act.Sqrt, bias=eps_sb[:]
                )
                rstd = spool.tile([128, 1], f32)
                nc.vector.reciprocal(out=rstd[:], in_=sd[:])
                # normalize in place on ACT: gt[:, c] = gt[:, c]*rstd - mean*rstd
                # (DVE carries bn_stats; ACT has headroom for the applies)
                beta = spool.tile([128, 1], f32)
                nc.vector.tensor_scalar(
                    out=beta[:],
                    in0=mv[:, 0:1],
                    scalar1=rstd[:],
                    scalar2=-1.0,
                    op0=AluOp.mult,
                    op1=AluOp.mult,
                )
                nc.scalar.activation(
                    out=gt[:, c],
                    in_=gt[:, c],
                    func=Act.Identity,
                    bias=beta[:],
                    scale=rstd[:],
                )
                if affine:
                    nc.vector.tensor_mul(gt[:, c], gt[:, c], sc_sb[:])
                    nc.vector.tensor_add(gt[:, c], gt[:, c], bi_sb[:])
                if c % 2 == 1:
                    # store per pair of chunks so output DMA overlaps compute
                    dst = bass.AP(
                        out_d,
                        (g * GROUP + (c - 1) * CHUNK) * HIDDEN,
                        [[HIDDEN, CHUNK], [CHUNK * HIDDEN, 2], [1, HIDDEN]],
                    )
                    nc.sync.dma_start(out=dst, in_=gt[:, c - 1 : c + 1, :])

    nc.compile()
    return nc


_kernel_cache: dict = {}
last_results = None


def _get_nc(tok_per_core: int, affine: bool):
    key = (tok_per_core, affine)
    if key not in _kernel_cache:
        _kernel_cache[key] = _build(tok_per_core, affine)
    return _kernel_cache[key]


def _fuse_tables(tables: np.ndarray) -> np.ndarray:
    """F[m] = concat_h T_h[(m * p_h) % 16384], with an extra row 16384 == row 0
    so the device-side index (id & 16383) + 1 needs no second mod."""
    m = np.arange(NUM_BUCKETS, dtype=np.int64)
    ftab = np.empty((NUM_BUCKETS + 1, HIDDEN), np.float32)
    for h in range(NUM_HASHES):
        hashed = (m * PRIMES[h]) % NUM_BUCKETS
        ftab[:NUM_BUCKETS, h * SHARD : (h + 1) * SHARD] = tables[h][hashed]
    ftab[NUM_BUCKETS] = ftab[0]
    return ftab


def _prep_inputs(input_ids, tables, ln_scale, ln_bias):
    input_ids = np.asarray(input_ids)
    tables = np.asarray(tables, dtype=np.float32)
    ln_scale = np.asarray(ln_scale, dtype=np.float32)
    ln_bias = np.asarray(ln_bias, dtype=np.float32)
    B, S = input_ids.shape
    tok_per_core = B * S // N_CORES
    affine = not (np.all(ln_scale == 1.0) and np.all(ln_bias == 0.0))

    # Note: F is indexed by (id+1) mod 16384; the reference hash is
    # ((id+1)*p) % 16384 and row F[(id+1)%16384] holds exactly those rows.
    ftab = _fuse_tables(tables)

    ids_flat = input_ids.reshape(-1).astype(np.int64).astype(np.int32)
    in_maps = []
    for c in range(N_CORES):
        idc = ids_flat[c * tok_per_core : (c + 1) * tok_per_core]
        # wrapped-16 layout: w16[p, g, s] = idc[g*GROUP + s*16 + p], replicated
        # over the 8 gpsimd-core partition groups
        w16 = idc.reshape(-1, GROUP // 16, 16).transpose(2, 0, 1)  # [16, g, s]
        w = np.tile(w16, (8, 1, 1)).reshape(128, -1)
        m = {"ids": np.ascontiguousarray(w), "ftab": ftab}
        if affine:
            m["ln_scale"] = np.ascontiguousarray(
                np.broadcast_to(ln_scale[None], (128, HIDDEN))
            )
            m["ln_bias"] = np.ascontiguousarray(
                np.broadcast_to(ln_bias[None], (128, HIDDEN))
            )
        in_maps.append(m)
    return in_maps, tok_per_core, affine, (B, S)


def kernel(input_ids, tables, ln_scale, ln_bias):
    global last_results
    in_maps, tok_per_core, affine, (B, S) = _prep_inputs(
        input_ids, tables, ln_scale, ln_bias
    )
    nc = _get_nc(tok_per_core, affine)
    res = run_bass_kernel_spmd(nc, in_maps, core_ids=list(range(N_CORES)))
    last_results = res
    out = np.stack([r["out"] for r in res.results], axis=0)
    return out.reshape(B, S, HIDDEN)


# revision 35
# speedup vs baseline: 1.1424x; 1.1424x over previous
"""CanineEmbeddings (multi-hash bucket embedding lookup + LayerNorm) on 8 TRN2 cores.

Key observation: every bucket hash ((id+1)*prime_h) % 16384 depends only on
m = (id+1) mod 16384, so there are exactly 16384 distinct embedding vectors.
The host fuses the 8 bucket tables into one table F[m] = concat_h T_h[(m*p_h)%16384]
(pure weight preprocessing), and the device does ONE 3072-byte dma_gather per
token instead of eight 384-byte ones.

Per-core structure (data-parallel over batch; one 8192-token row per core):
  - ids arrive wrapped-16 ([p, g, s] = id[g*1024 + s*16 + p%16], replicated
    across the 8 gpsimd core groups) so the SWDGE gather can read them.
  - idx = (id & 16383) + 1 on DVE (2 ops); F has 16385 rows with row 16384
    aliasing row 0 so the +1 never needs a second mod.
  - per 1024-token group: one dma_gather (SWDGE 'mlp' Q7 library, 4 queues
    round-robin) -> gt[p, chunk, 768] is already the packed layout.
  - LayerNorm per 128-token chunk: 2x bn_stats (384 elems each) + bn_aggr +
    sqrt + reciprocal; normalize in-place, alternating ACT and DVE.
  - store per 2 chunks (HWDGE, 3072B descriptors).
"""

import contextlib
import ctypes
import os
import sys
import types

import numpy as np

import concourse.bacc as bacc
import concourse.bass as bass
import concourse.mybir as mybir
import concourse.tile as tile
from concourse.bass_utils import run_bass_kernel_spmd
from concourse.library_config import mlp as _mlp_lib
from concourse.tile import add_dep_helper


def _ensure_axon_ntff_hook():
    """The agent image's ``antenv`` lacks ``axon_hooks``; provide it (and the
    ctypes NTFF profile hook) so run_bass_kernel_spmd(trace=True) works.
    Degrades to a None hook (no trace, run still works) on any failure."""
    if "antenv.axon_hooks" in sys.modules:
        return
    hook = None
    try:
        so_path = "/opt/axon/libaxon_pjrt.so"
        lib = ctypes.CDLL(so_path)
        if hasattr(lib, "axon_start_nrt_profile"):
            lib.axon_start_nrt_profile.argtypes = [
                ctypes.POINTER(ctypes.c_int64),
                ctypes.c_size_t,
            ]
            lib.axon_start_nrt_profile.restype = ctypes.c_int64
            lib.axon_stop_nrt_profile.argtypes = [ctypes.c_char_p]
            lib.axon_stop_nrt_profile.restype = ctypes.c_int64

            @contextlib.contextmanager
            def _hook(output_dir, device_ids):
                import jax

                jax.devices()
                if device_ids:
                    ids = (ctypes.c_int64 * len(device_ids))(*device_ids)
                    rc = lib.axon_start_nrt_profile(ids, len(device_ids))
                else:
                    rc = lib.axon_start_nrt_profile(None, 0)
                if rc != 0:
                    raise RuntimeError(f"axon_start_nrt_profile rc={rc}")
                try:
                    yield
                finally:
                    n = lib.axon_stop_nrt_profile(str(output_dir).encode())
                    print(f"ntff profile: {n} file(s) -> {output_dir}", file=sys.stderr)

            hook = _hook
    except Exception as e:  # pragma: no cover
        print(f"ntff hook unavailable: {e}", file=sys.stderr)
    mod = types.ModuleType("antenv.axon_hooks")
    mod.get_axon_ntff_profile_hook = lambda: hook
    mod.set_axon_ntff_profile_hook = lambda h: None
    sys.modules["antenv.axon_hooks"] = mod


_ensure_axon_ntff_hook()

PRIMES = [31, 43, 59, 61, 73, 97, 103, 113]
NUM_HASHES = 8
NUM_BUCKETS = 16384
HIDDEN = 768
SHARD = 96
LN_EPS = 1e-6
N_CORES = 8
GROUP = 1024  # tokens per gather (SWDGE ring caps one gather at 1024 descriptors)
CHUNK = 128  # tokens per LayerNorm chunk (one partition sweep)

AluOp = mybir.AluOpType
Act = mybir.ActivationFunctionType


def _build(tok_per_core: int, affine: bool, enable_asserts: bool = False):
    n_groups = tok_per_core // GROUP
    n_chunks = GROUP // CHUNK  # 8
    wrap_s = GROUP // 16  # 64
    f32, i32, i16 = mybir.dt.float32, mybir.dt.int32, mybir.dt.int16

    nc = bacc.Bacc(
        "TRN2",
        target_bir_lowering=False,
        debug=False,
        enable_asserts=enable_asserts,
        # dma_gather desc-gen runs on the Q7 cpu pair selected by queue_num;
        # 4 queues let up to 4 gathers generate descriptors concurrently.
        num_swdge_queues=4,
    )

    ids_d = nc.dram_tensor("ids", [128, n_groups * wrap_s], i32, kind="ExternalInput")
    ftab_d = nc.dram_tensor(
        "ftab", [NUM_BUCKETS + 1, HIDDEN], f32, kind="ExternalInput"
    )
    out_d = nc.dram_tensor("out", [tok_per_core, HIDDEN], f32, kind="ExternalOutput")
    if affine:
        sc_d = nc.dram_tensor("ln_scale", [128, HIDDEN], f32, kind="ExternalInput")
        bi_d = nc.dram_tensor("ln_bias", [128, HIDDEN], f32, kind="ExternalInput")

    from contextlib import ExitStack

    with tile.TileContext(nc) as tc, ExitStack() as ctx:
        # dma_gather is a Q7 extended instruction living in the 'mlp' ucode
        # library; it must be loaded on the Pool engine before any gather.
        lib_inst = nc.gpsimd.load_library(_mlp_lib).ins

        const = ctx.enter_context(tc.tile_pool(name="const", bufs=1))
        gpool = ctx.enter_context(tc.tile_pool(name="gather", bufs=4))
        spool = ctx.enter_context(tc.tile_pool(name="stats", bufs=8))

        eps_sb = const.tile([128, 1], f32)
        nc.vector.memset(eps_sb[:], LN_EPS)

        ids_sb = const.tile([128, n_groups, wrap_s], i32)
        nc.sync.dma_start(
            out=ids_sb[:],
            in_=ids_d[:].rearrange("p (g s) -> p g s", g=n_groups),
        )
        if affine:
            sc_sb = const.tile([128, HIDDEN], f32)
            nc.sync.dma_start(out=sc_sb[:], in_=sc_d[:])
            bi_sb = const.tile([128, HIDDEN], f32)
            nc.sync.dma_start(out=bi_sb[:], in_=bi_d[:])

        # idx = (id & 16383) + 1 in [1, 16384]; F row 16384 aliases row 0.
        # DVE arithmetic runs in fp32, but all values stay < 2^24 so exact.
        m_sb = const.tile([128, n_groups, wrap_s], i32)
        nc.vector.tensor_scalar(
            out=m_sb[:],
            in0=ids_sb[:],
            scalar1=NUM_BUCKETS - 1,
            scalar2=None,
            op0=AluOp.bitwise_and,
        )
        idx_all = const.tile([128, n_groups, wrap_s], i16)
        nc.vector.tensor_scalar(
            out=idx_all[:],
            in0=m_sb[:],
            scalar1=1,
            scalar2=None,
            op0=AluOp.add,
        )

        for g in range(n_groups):
            # gt[p, chunk, 0:768]: token (g*1024 + chunk*128 + p)'s full
            # (pre-LayerNorm) embedding, gathered in packed layout.
            gt = gpool.tile([128, n_chunks, HIDDEN], f32)
            gi = nc.gpsimd.dma_gather(
                out_ap=gt[:],
                in_ap=ftab_d[:],
                idxs_ap=idx_all[:, g, :],
                num_idxs=GROUP,
                num_idxs_reg=GROUP,
                elem_size=HIDDEN,
                queue_num=g % 4,
            )
            add_dep_helper(gi.ins, lib_inst, sync=False, reason="needs mlp lib")

            for c in range(n_chunks):
                stats = spool.tile([128, 2, 6], f32)
                nc.vector.bn_stats(out=stats[:, 0, :], in_=gt[:, c, 0 : HIDDEN // 2])
                nc.vector.bn_stats(out=stats[:, 1, :], in_=gt[:, c, HIDDEN // 2 :])
                mv = spool.tile([128, 2], f32)
                nc.vector.bn_aggr(out=mv[:], in_=stats[:])
                sd = spool.tile([128, 1], f32)
                nc.scalar.activation(
                    out=sd[:], in_=mv[:, 1:2], func=Act.Sqrt, bias=eps_sb[:]
                )
                rstd = spool.tile([128, 1], f32)
                nc.vector.reciprocal(out=rstd[:], in_=sd[:])
                # normalize in place on ACT: gt[:, c] = gt[:, c]*rstd - mean*rstd
                # (DVE carries bn_stats; ACT has headroom for the applies)
                beta = spool.tile([128, 1], f32)
                nc.vector.tensor_scalar(
                    out=beta[:],
                    in0=mv[:, 0:1],
                    scalar1=rstd[:],
                    scalar2=-1.0,
                    op0=AluOp.mult,
                    op1=AluOp.mult,
                )
                nc.scalar.activation(
                    out=gt[:, c],
                    in_=gt[:, c],
                    func=Act.Identity,
                    bias=beta[:],
                    scale=rstd[:],
                )
                if affine:
                    nc.vector.tensor_mul(gt[:, c], gt[:, c], sc_sb[:])
                    nc.vector.tensor_add(gt[:, c], gt[:, c], bi_sb[:])
                if c % 4 == 3:
                    # store per 4 chunks so output DMA overlaps compute
                    dst = bass.AP(
                        out_d,
                        (g * GROUP + (c - 3) * CHUNK) * HIDDEN,
                        [[HIDDEN, CHUNK], [CHUNK * HIDDEN, 4], [1, HIDDEN]],
                    )
                    nc.sync.dma_start(out=dst, in_=gt[:, c - 3 : c + 1, :])

    nc.compile()
    return nc


_kernel_cache: dict = {}
last_results = None


def _get_nc(tok_per_core: int, affine: bool):
    key = (tok_per_core, affine)
    if key not in _kernel_cache:
        _kernel_cache[key] = _build(tok_per_core, affine)
    return _kernel_cache[key]


def _fuse_tables(tables: np.ndarray) -> np.ndarray:
    """F[m] = concat_h T_h[(m * p_h) % 16384], with an extra row 16384 == row 0
    so the device-side index (id & 16383) + 1 needs no second mod."""
    m = np.arange(NUM_BUCKETS, dtype=np.int64)
    ftab = np.empty((NUM_BUCKETS + 1, HIDDEN), np.float32)
    for h in range(NUM_HASHES):
        hashed = (m * PRIMES[h]) % NUM_BUCKETS
        ftab[:NUM_BUCKETS, h * SHARD : (h + 1) * SHARD] = tables[h][hashed]
    ftab[NUM_BUCKETS] = ftab[0]
    return ftab


def _prep_inputs(input_ids, tables, ln_scale, ln_bias):
    input_ids = np.asarray(input_ids)
    tables = np.asarray(tables, dtype=np.float32)
    ln_scale = np.asarray(ln_scale, dtype=np.float32)
    ln_bias = np.asarray(ln_bias, dtype=np.float32)
    B, S = input_ids.shape
    tok_per_core = B * S // N_CORES
    affine = not (np.all(ln_scale == 1.0) and np.all(ln_bias == 0.0))

    # Note: F is indexed by (id+1) mod 16384; the reference hash is
    # ((id+1)*p) % 16384 and row F[(id+1)%16384] holds exactly those rows.
    ftab = _fuse_tables(tables)

    ids_flat = input_ids.reshape(-1).astype(np.int64).astype(np.int32)
    in_maps = []
    for c in range(N_CORES):
        idc = ids_flat[c * tok_per_core : (c + 1) * tok_per_core]
        # wrapped-16 layout: w16[p, g, s] = idc[g*GROUP + s*16 + p], replicated
        # over the 8 gpsimd-core partition groups
        w16 = idc.reshape(-1, GROUP // 16, 16).transpose(2, 0, 1)  # [16, g, s]
        w = np.tile(w16, (8, 1, 1)).reshape(128, -1)
        m = {"ids": np.ascontiguousarray(w), "ftab": ftab}
        if affine:
            m["ln_scale"] = np.ascontiguousarray(
                np.broadcast_to(ln_scale[None], (128, HIDDEN))
            )
            m["ln_bias"] = np.ascontiguousarray(
                np.broadcast_to(ln_bias[None], (128, HIDDEN))
            )
        in_maps.append(m)
    return in_maps, tok_per_core, affine, (B, S)


def kernel(input_ids, tables, ln_scale, ln_bias):
    global last_results
    in_maps, tok_per_core, affine, (B, S) = _prep_inputs(
        input_ids, tables, ln_scale, ln_bias
    )
    nc = _get_nc(tok_per_core, affine)
    res = run_bass_kernel_spmd(nc, in_maps, core_ids=list(range(N_CORES)))
    last_results = res
    out = np.stack([r["out"] for r in res.results], axis=0)
    return out.reshape(B, S, HIDDEN)
